# revision 45
# baseline (speedup 1.0000x reference)
"""Trainium2 Bass kernel for nn_Encoding (vq_codebook), bf16 restructure.

Math (per batch b):
    xf = x[b].reshape(C, N).T                      # (N tokens, C)
    sl2[n,k] = scale[k] * (|xf_n|^2 - 2 xf_n.c_k + |c_k|^2)
    w = softmax_k(sl2)
    out[b] = w.T @ xf - (sum_n w)[:,None] * codewords

Sharding: data-parallel over batch B=32 -> 4 batches per core on 8 cores.

Key idea vs the fp32 predecessor: the host ships x twice in bf16 --
natural layout (channel-partition, for mm1) AND pre-transposed layout
(token-partition, for mm2) -- same 16 MiB/core of HBM traffic as one
fp32 copy, but zero on-device PE transposes of x and no PSUM
evacuation pipeline. All PE matmuls on x are bf16 single-pass (the
fp32 path compiles to LOW_HIGH two-pass); |x|^2 and its fold into the
logits stay fp32. Verified numerically: full-bf16 rel err 2.8e-3 vs
2e-2 tolerance.

Per-core dataflow (unit = 2048 tokens; 2 units/batch, 8 units/core):
  - mm1: psl2 (128 = 4 token-groups x 32 codes, 512 tokens) accumulates
    A = -2*scale*cw (bf16) against streamed natural-layout x.
  - |x|^2 per token from the transposed tiles: even chunks via 4
    grouped DVE bn_stats (exact fp32 moments of the bf16 values), odd
    chunks via ACT Square+accum_out -> xsqT (128,16) fp32; PE-transposed
    (fp32) + DRAM-bounced to (4,512); a rank-4 fp32 matmul adds
    scale_k*|x|^2 into the same PSUM.
  - One ACT exp over (128,512) with per-partition bias
    scale_k*|c_k|^2 + 8 (the +8 cancels in the softmax; keeps e away
    from bf16 underflow), output bf16.
  - Softmax denominators: PE group-indicator matmul -> (4,512); DVE
    reciprocal_approx_fast; PE broadcast back to (128,512) fp32; DVE
    multiply normalizes -> w (bf16).
  - PE transposes w into (token, code) tiles; mm2 (w stationary, xT
    moving, bf16) accumulates out (32, 258) per batch; col 256 of xT
    is ones (wsum rides the same PSUM), col 257 zero pad.
  - Final: one DVE scalar_tensor_tensor: out = (-cw)*wsum + wx; DMA.
"""

import numpy as np
import ml_dtypes
from contextlib import ExitStack

import concourse.bass as bass
import concourse.bacc as bacc
import concourse.mybir as mybir
import concourse.tile as tile
from concourse.bass_utils import run_bass_kernel_spmd

F32 = mybir.dt.float32
F16 = mybir.dt.float16
BF16 = mybir.dt.bfloat16
FP8 = mybir.dt.float8e4
ALU = mybir.AluOpType
ACTF = mybir.ActivationFunctionType
BF = ml_dtypes.bfloat16
F8 = ml_dtypes.float8_e4m3fn
ASCALE = 256.0          # fp8 rescale of A; undone in the exp's scale

N_CORES = 8
B, C, K = 32, 256, 32
HW = 64 * 64            # 4096 tokens per batch
BL = B // N_CORES       # batches per core
UNIT = 2048             # tokens per unit
NGRP = 4                # 512-token groups per unit
GTOK = 512              # tokens per group
NCHUNK = 16             # 128-token chunks per unit
XTW = 258               # xT chunk width: 256 ch + ones + pad


def build_module(bl=BL):
    nc = bacc.Bacc(None)
    units = bl * HW // UNIT

    xn_d = nc.dram_tensor("XN", (bl, 128, 2, HW), FP8, kind="ExternalInput")
    xt_d = nc.dram_tensor("XT", (units, 128, NCHUNK * XTW), BF16,
                          kind="ExternalInput")
    a_d = nc.dram_tensor("A", (NGRP, 128, 2, 128), FP8, kind="ExternalInput")
    sg_d = nc.dram_tensor("SG", (NGRP, 128, 128), F16, kind="ExternalInput")
    bias_d = nc.dram_tensor("BIASB", (128, 1), F32, kind="ExternalInput")
    gs_d = nc.dram_tensor("GS", (128, 4), BF16, kind="ExternalInput")
    gb_d = nc.dram_tensor("GB", (4, 128), BF16, kind="ExternalInput")
    cw_d = nc.dram_tensor("CWD", (32, 256), F32, kind="ExternalInput")
    idt_d = nc.dram_tensor("IDT", (128, 128), BF16, kind="ExternalInput")
    out_d = nc.dram_tensor("out", (bl, 32, 256), F32, kind="ExternalOutput")

    with tile.TileContext(nc) as tc, ExitStack() as ctx:
        sb = ctx.enter_context(tc.tile_pool(name="sb", bufs=2))
        sbx = ctx.enter_context(tc.tile_pool(name="sbx", bufs=3))
        cp = ctx.enter_context(tc.tile_pool(name="consts", bufs=1))
        ps_big = ctx.enter_context(tc.tile_pool(name="ps_big", bufs=2, space="PSUM"))
        ps_sm = ctx.enter_context(tc.tile_pool(name="ps_sm", bufs=1, space="PSUM"))
        ps_pr = ctx.enter_context(tc.tile_pool(name="ps_pr", bufs=1, space="PSUM"))
        ps_wtt = ctx.enter_context(tc.tile_pool(name="ps_wtt", bufs=1, space="PSUM"))
        ps_wx = ctx.enter_context(tc.tile_pool(name="ps_wx", bufs=1, space="PSUM"))
        dr = ctx.enter_context(tc.tile_pool(name="dr", bufs=2, space="DRAM"))

        def c(shape, dram, tag, dt=F32):
            t = cp.tile(shape, dt, tag=tag)
            nc.sync.dma_start(t[:], dram[:])
            return t

        def load_consts():
            nonlocal a_s, sg_s, bias_s, gs_s, gb_s, cw_s, idt_s
            a_s = cp.tile([128, NGRP, 2, 128], FP8, tag="a")
            nc.sync.dma_start(a_s[:], a_d[:].rearrange("g p h m -> p g h m"))
            sg_s = cp.tile([128, NGRP, 128], F16, tag="sg")
            nc.sync.dma_start(sg_s[:], sg_d[:].rearrange("g p m -> p g m"))
            bias_s = c([128, 1], bias_d, "bias")
            gs_s = c([128, 4], gs_d, "gs", BF16)
            gb_s = c([4, 128], gb_d, "gb", BF16)
            cw_s = c([32, 256], cw_d, "cw")
            idt_s = c([128, 128], idt_d, "idt", BF16)

        a_s = sg_s = bias_s = gs_s = gb_s = cw_s = idt_s = None
        pwx = {}

        def stage_load(u):
            """Issue the unit's DMA loads (runs ~2 units ahead)."""
            b_, uu = u // 2, u % 2
            t0 = uu * UNIT
            xn = sbx.tile([128, 2, UNIT], FP8, tag="xn")
            nc.sync.dma_start(xn[:], xn_d[b_, :, :, t0:t0 + UNIT])
            xT = sbx.tile([128, NCHUNK * XTW], BF16, tag="xT")
            nc.sync.dma_start(xT[:], xt_d[u])
            return dict(xn=xn, xT=xT, b=b_, uu=uu, u=u)

        def stage_a(st):
            """squares + mm1 (xc, scale*|x|^2 ones-fold) into psl2."""
            xn = st["xn"]

            # x^2 from the SAME fp8 values mm1 contracts -- a consistent
            # perturbed problem (verified 3.5e-3). fp8^2 has <=7 mantissa
            # bits, so the fp16 sq tile is exact; the channel-half pre-sum
            # (DVE add, exact in fp16? values <~50, 2^-11 rel -- fine)
            # halves the PE ones-fold to 4 matmuls.
            sq = sb.tile([128, 2, UNIT], F16, tag="sq")
            sq2 = sb.tile([128, UNIT], F16, tag="sq2")
            for th in (0, 1):
                ts_ = slice(th * (UNIT // 2), (th + 1) * (UNIT // 2))
                nc.scalar.activation(sq[:, :, ts_], xn[:, :, ts_], ACTF.Square)
                nc.vector.tensor_tensor(
                    sq2[:, ts_], sq[:, 0, ts_], sq[:, 1, ts_], ALU.add)

            psl2 = ps_big.tile([128, 512], F32, tag="big")
            for g in range(NGRP):
                # fp8 DoubleRow: contract both 128-channel halves at once
                nc.tensor.matmul(
                    psl2[:, :],
                    a_s[:, g, :, :],
                    xn[:, :, g * GTOK:(g + 1) * GTOK],
                    start=(g == 0), stop=False, skip_group_check=True,
                    perf_mode=mybir.MatmulPerfMode.DoubleRow,
                )
            # scale_k*|x|^2 via ones-style fold: SG[c, 32g+k] = 256*scale_k
            # (zero outside group g's columns), contracting pre-summed x^2.
            for g in range(NGRP):
                nc.tensor.matmul(
                    psl2[:, :],
                    sg_s[:, g, :],
                    sq2[:, g * GTOK:(g + 1) * GTOK],
                    start=False, stop=(g == NGRP - 1),
                    skip_group_check=True,
                )
            return dict(psl2=psl2, xT=st["xT"], b=st["b"], uu=st["uu"],
                        u=st["u"])

        def stage_b(st):
            """softmax + mm2 + (end of batch) final subtract + store."""
            psl2, xT, b_, uu = st["psl2"], st["xT"], st["b"], st["uu"]
            e = sb.tile([128, 512], BF16, tag="e")
            nc.scalar.activation(e[:], psl2[:], ACTF.Exp, bias=bias_s[:],
                                 scale=1.0 / ASCALE)
            ps4 = ps_sm.tile([4, 512], F32, tag="sm")
            nc.tensor.matmul(ps4[:], gs_s[:], e[:])
            # ~18-bit reciprocal straight to bf16 (wrapper insists on fp32
            # out; the NR result casts on the write port)
            from concourse.dve_ops import (
                RECIP_APPROX_FAST_CONSTS as _RC,
                RECIPROCAL_APPROX_FAST as _RF,
            )
            r4 = sb.tile([4, 512], BF16, tag="r4")
            nc.vector._custom_dve(
                _RF, out=r4[:], in0=ps4[:],
                s0=_RC["s0"], s1=_RC["s1"], imm2=_RC["imm2"],
            )
            pR = ps_pr.tile([128, 512], F32, tag="pr")
            nc.tensor.matmul(pR[:], gb_s[:], r4[:])
            wt = sb.tile([128, 512], BF16, tag="wt")
            nc.vector.tensor_tensor(wt[:], e[:], pR[:], ALU.mult)

            if uu == 0:
                pwx[b_] = ps_wx.tile([32, XTW], F32, tag="wx", name="pwx")

            pwtT = ps_wtt.tile([128, 512], BF16, tag="wtt")
            for sl in range(4):
                nc.tensor.transpose(
                    pwtT[:, 128 * sl:128 * sl + 128],
                    wt[:, 128 * sl:128 * sl + 128],
                    idt_s[:],
                )
            wtTs = sb.tile([128, 512], BF16, tag="wtTs")
            nc.vector.tensor_copy(wtTs[:], pwtT[:])
            for j in range(NCHUNK):
                nc.tensor.matmul(
                    pwx[b_][:, 0:XTW],
                    wtTs[:, 128 * (j % 4) + 32 * (j // 4):
                         128 * (j % 4) + 32 * (j // 4) + 32],
                    xT[:, XTW * j:XTW * (j + 1)],
                    start=(uu == 0 and j == 0),
                    stop=(uu == 1 and j == NCHUNK - 1),
                    skip_group_check=True,
                )
            if uu == 1:
                outs = sb.tile([32, 256], F32, tag="outs")
                nc.vector.scalar_tensor_tensor(
                    out=outs[:], in0=cw_s[:], scalar=pwx[b_][:, 256:257],
                    in1=pwx[b_][:, 0:256], op0=ALU.mult, op1=ALU.add,
                )
                nc.gpsimd.dma_start(out_d[b_], outs[:])
                del pwx[b_]

        # unit-0/1 loads hit the DMA queue before the constants (the x
        # data is the critical path at startup); loads then run 2 units
        # ahead, and stage_b(u-1) is emitted before stage_a(u) so the
        # softmax chain outranks the next unit's work in every engine's
        # priority queue.
        loads = [stage_load(0), stage_load(1)]
        load_consts()
        loads.append(stage_load(2))
        prev = stage_a(loads[0])
        for u in range(1, units):
            if u + 2 < units:
                loads.append(stage_load(u + 2))
            stage_b(prev)
            prev = stage_a(loads[u])
        stage_b(prev)

    nc.finalize()
    return nc


def host_constants(codewords, scale):
    cw = np.asarray(codewords, dtype=np.float32)
    sc = np.asarray(scale, dtype=np.float32)
    c_sq = (cw.astype(np.float64) ** 2).sum(-1).astype(np.float32)

    # A[g, p, h, m]: fp8 DoubleRow layout — contraction pair (p, h)
    # covers channel h*128+p; rescaled by ASCALE for e4m3 range.
    A = np.zeros((NGRP, 128, 2, 128), np.float32)
    for cc in range(2):
        blk = ASCALE * (-2.0 * sc[None, :]) * cw[:, cc * 128:(cc + 1) * 128].T
        for g in range(NGRP):
            A[g, :, cc, 32 * g:32 * g + 32] = blk

    SG = np.zeros((NGRP, 128, 128), np.float32)
    BIASB = np.zeros((128, 1), np.float32)
    GS = np.zeros((128, 4), np.float32)
    GB = np.zeros((4, 128), np.float32)
    for g in range(4):
        SG[g, :, 32 * g:32 * g + 32] = (ASCALE * sc)[None, :]
        BIASB[32 * g:32 * g + 32, 0] = sc * c_sq + 8.0
        GS[32 * g:32 * g + 32, g] = 1.0
        GB[g, 32 * g:32 * g + 32] = 1.0

    return {
        "A": A.astype(F8), "SG": SG.astype(np.float16), "BIASB": BIASB,
        "GS": GS.astype(BF), "GB": GB.astype(BF),
        "CWD": np.ascontiguousarray(-cw),
        "IDT": np.eye(128, dtype=BF),
    }


_CACHE = {}


def pack_x(x):
    """Host marshaling: bf16 natural + bf16 pre-transposed layouts."""
    xb = x.reshape(B, 2, 128, HW).astype(BF)        # (b, cc, p, t)
    xn = np.ascontiguousarray(
        x.reshape(B, 2, 128, HW).astype(F8).transpose(0, 2, 1, 3))
    # transposed: (b, chunk, i, c) with ones/pad cols, then unit-major
    xt = np.empty((B, HW // 128, 128, XTW), dtype=BF)
    xt[..., 256] = 1.0
    xt[..., 257] = 0.0
    # (b, cc, p, ch, i) -> (b, ch, i, cc*128+p)
    xt[..., 0:256] = (
        xb.reshape(B, 2, 128, HW // 128, 128)
        .transpose(0, 3, 4, 1, 2)
        .reshape(B, HW // 128, 128, 256))
    # (b, ch, i, c) -> (unit, j, i, c) -> (unit, i, j*c)
    xt = xt.reshape(B * HW // UNIT, NCHUNK, 128, XTW).transpose(0, 2, 1, 3)
    xt = np.ascontiguousarray(xt.reshape(B * HW // UNIT, 128, NCHUNK * XTW))
    return xn, xt


def make_in_maps(inputs):
    x = np.asarray(inputs["x"], dtype=np.float32)
    consts = host_constants(inputs["codewords"], inputs["scale"])
    xn, xt = pack_x(x)
    upc = BL * HW // UNIT   # units per core
    in_maps = []
    for i in range(N_CORES):
        m = dict(consts)
        m["XN"] = np.ascontiguousarray(xn[BL * i:BL * (i + 1)])
        m["XT"] = np.ascontiguousarray(xt[upc * i:upc * (i + 1)])
        in_maps.append(m)
    return in_maps


def kernel(x, codewords, scale):
    if "nc" not in _CACHE:
        _CACHE["nc"] = build_module()
    nc = _CACHE["nc"]
    in_maps = make_in_maps(dict(x=x, codewords=codewords, scale=scale))
    res = run_bass_kernel_spmd(nc, in_maps, list(range(N_CORES)))
    out = np.concatenate([r["out"] for r in res.results], axis=0)
    return out.astype(np.float32)


# revision 49
# speedup vs baseline: 1.1390x; 1.1390x over previous
"""Trainium2 Bass kernel for nn_Encoding (vq_codebook), bf16 restructure.

Math (per batch b):
    xf = x[b].reshape(C, N).T                      # (N tokens, C)
    sl2[n,k] = scale[k] * (|xf_n|^2 - 2 xf_n.c_k + |c_k|^2)
    w = softmax_k(sl2)
    out[b] = w.T @ xf - (sum_n w)[:,None] * codewords

Sharding: data-parallel over batch B=32 -> 4 batches per core on 8 cores.

Key idea vs the fp32 predecessor: the host ships x twice in bf16 --
natural layout (channel-partition, for mm1) AND pre-transposed layout
(token-partition, for mm2) -- same 16 MiB/core of HBM traffic as one
fp32 copy, but zero on-device PE transposes of x and no PSUM
evacuation pipeline. All PE matmuls on x are bf16 single-pass (the
fp32 path compiles to LOW_HIGH two-pass); |x|^2 and its fold into the
logits stay fp32. Verified numerically: full-bf16 rel err 2.8e-3 vs
2e-2 tolerance.

Per-core dataflow (unit = 2048 tokens; 2 units/batch, 8 units/core):
  - mm1: psl2 (128 = 4 token-groups x 32 codes, 512 tokens) accumulates
    A = -2*scale*cw (bf16) against streamed natural-layout x.
  - |x|^2 per token from the transposed tiles: even chunks via 4
    grouped DVE bn_stats (exact fp32 moments of the bf16 values), odd
    chunks via ACT Square+accum_out -> xsqT (128,16) fp32; PE-transposed
    (fp32) + DRAM-bounced to (4,512); a rank-4 fp32 matmul adds
    scale_k*|x|^2 into the same PSUM.
  - One ACT exp over (128,512) with per-partition bias
    scale_k*|c_k|^2 + 8 (the +8 cancels in the softmax; keeps e away
    from bf16 underflow), output bf16.
  - Softmax denominators: PE group-indicator matmul -> (4,512); DVE
    reciprocal_approx_fast; PE broadcast back to (128,512) fp32; DVE
    multiply normalizes -> w (bf16).
  - PE transposes w into (token, code) tiles; mm2 (w stationary, xT
    moving, bf16) accumulates out (32, 258) per batch; col 256 of xT
    is ones (wsum rides the same PSUM), col 257 zero pad.
  - Final: one DVE scalar_tensor_tensor: out = (-cw)*wsum + wx; DMA.
"""

import numpy as np
import ml_dtypes
from contextlib import ExitStack

import concourse.bass as bass
import concourse.bacc as bacc
import concourse.mybir as mybir
import concourse.tile as tile
from concourse.bass_utils import run_bass_kernel_spmd

F32 = mybir.dt.float32
F16 = mybir.dt.float16
BF16 = mybir.dt.bfloat16
FP8 = mybir.dt.float8e4
ALU = mybir.AluOpType
ACTF = mybir.ActivationFunctionType
BF = ml_dtypes.bfloat16
F8 = ml_dtypes.float8_e4m3fn
ASCALE = 256.0          # fp8 rescale of A; undone in the exp's scale

N_CORES = 8
B, C, K = 32, 256, 32
HW = 64 * 64            # 4096 tokens per batch
BL = B // N_CORES       # batches per core
UNIT = 2048             # tokens per unit
NGRP = 4                # 512-token groups per unit
GTOK = 512              # tokens per group
NCHUNK = 16             # 128-token chunks per unit
XTW = 258               # xT chunk width: 256 ch + ones + pad


def build_module(bl=BL):
    nc = bacc.Bacc(None)
    units = bl * HW // UNIT

    xn_d = nc.dram_tensor("XN", (bl, 128, 2, HW), FP8, kind="ExternalInput")
    xt_d = nc.dram_tensor("XT", (units, 128, NCHUNK * XTW), BF16,
                          kind="ExternalInput")
    a_d = nc.dram_tensor("A", (128, NGRP, 2, 128), FP8, kind="ExternalInput")
    sg_d = nc.dram_tensor("SG", (128, NGRP, 128), F16, kind="ExternalInput")
    bias_d = nc.dram_tensor("BIASB", (128, 1), F32, kind="ExternalInput")
    gs_d = nc.dram_tensor("GS", (128, 4), BF16, kind="ExternalInput")
    gb_d = nc.dram_tensor("GB", (4, 128), BF16, kind="ExternalInput")
    cw_d = nc.dram_tensor("CWD", (32, 256), F32, kind="ExternalInput")
    idt_d = nc.dram_tensor("IDT", (128, 128), BF16, kind="ExternalInput")
    out_d = nc.dram_tensor("out", (bl, 32, 256), F32, kind="ExternalOutput")

    with tile.TileContext(nc) as tc, ExitStack() as ctx:
        sb = ctx.enter_context(tc.tile_pool(name="sb", bufs=2))
        sbx = ctx.enter_context(tc.tile_pool(name="sbx", bufs=3))
        cp = ctx.enter_context(tc.tile_pool(name="consts", bufs=1))
        ps_big = ctx.enter_context(tc.tile_pool(name="ps_big", bufs=2, space="PSUM"))
        ps_sm = ctx.enter_context(tc.tile_pool(name="ps_sm", bufs=1, space="PSUM"))
        ps_pr = ctx.enter_context(tc.tile_pool(name="ps_pr", bufs=1, space="PSUM"))
        ps_wtt = ctx.enter_context(tc.tile_pool(name="ps_wtt", bufs=1, space="PSUM"))
        ps_wx = ctx.enter_context(tc.tile_pool(name="ps_wx", bufs=1, space="PSUM"))
        dr = ctx.enter_context(tc.tile_pool(name="dr", bufs=2, space="DRAM"))

        def c(shape, dram, tag, dt=F32):
            t = cp.tile(shape, dt, tag=tag)
            nc.sync.dma_start(t[:], dram[:])
            return t

        def load_consts():
            nonlocal a_s, sg_s, bias_s, gs_s, gb_s, cw_s, idt_s
            a_s = cp.tile([128, NGRP, 2, 128], FP8, tag="a")
            nc.sync.dma_start(a_s[:], a_d[:])
            sg_s = cp.tile([128, NGRP, 128], F16, tag="sg")
            nc.sync.dma_start(sg_s[:], sg_d[:])
            bias_s = c([128, 1], bias_d, "bias")
            gs_s = c([128, 4], gs_d, "gs", BF16)
            gb_s = c([4, 128], gb_d, "gb", BF16)
            cw_s = c([32, 256], cw_d, "cw")
            idt_s = c([128, 128], idt_d, "idt", BF16)

        a_s = sg_s = bias_s = gs_s = gb_s = cw_s = idt_s = None
        pwx = {}

        def stage_load(u):
            """Issue the unit's DMA loads (runs ~2 units ahead)."""
            b_, uu = u // 2, u % 2
            t0 = uu * UNIT
            xn = sbx.tile([128, 2, UNIT], FP8, tag="xn")
            nc.sync.dma_start(xn[:], xn_d[b_, :, :, t0:t0 + UNIT])
            xT = sbx.tile([128, NCHUNK * XTW], BF16, tag="xT")
            nc.sync.dma_start(xT[:], xt_d[u])
            return dict(xn=xn, xT=xT, b=b_, uu=uu, u=u)

        def stage_a(st):
            """squares + mm1 (xc, scale*|x|^2 ones-fold) into psl2."""
            xn = st["xn"]

            # x^2 from the SAME fp8 values mm1 contracts -- a consistent
            # perturbed problem (verified 3.5e-3). fp8^2 has <=7 mantissa
            # bits, so the fp16 sq tile is exact; the channel-half pre-sum
            # (DVE add, exact in fp16? values <~50, 2^-11 rel -- fine)
            # halves the PE ones-fold to 4 matmuls.
            sq = sb.tile([128, 2, UNIT], F16, tag="sq")
            sq2 = sb.tile([128, UNIT], F16, tag="sq2")
            for th in (0, 1):
                ts_ = slice(th * (UNIT // 2), (th + 1) * (UNIT // 2))
                nc.scalar.activation(sq[:, :, ts_], xn[:, :, ts_], ACTF.Square)
                nc.vector.tensor_tensor(
                    sq2[:, ts_], sq[:, 0, ts_], sq[:, 1, ts_], ALU.add)

            psl2 = ps_big.tile([128, 512], F32, tag="big")
            for g in range(NGRP):
                # fp8 DoubleRow: contract both 128-channel halves at once
                nc.tensor.matmul(
                    psl2[:, :],
                    a_s[:, g, :, :],
                    xn[:, :, g * GTOK:(g + 1) * GTOK],
                    start=(g == 0), stop=False, skip_group_check=True,
                    perf_mode=mybir.MatmulPerfMode.DoubleRow,
                )
            # scale_k*|x|^2 via ones-style fold: SG[c, 32g+k] = 256*scale_k
            # (zero outside group g's columns), contracting pre-summed x^2.
            for g in range(NGRP):
                nc.tensor.matmul(
                    psl2[:, :],
                    sg_s[:, g, :],
                    sq2[:, g * GTOK:(g + 1) * GTOK],
                    start=False, stop=(g == NGRP - 1),
                    skip_group_check=True,
                )
            return dict(psl2=psl2, xT=st["xT"], b=st["b"], uu=st["uu"],
                        u=st["u"])

        def stage_b(st):
            """softmax + mm2 + (end of batch) final subtract + store."""
            psl2, xT, b_, uu = st["psl2"], st["xT"], st["b"], st["uu"]
            e = sb.tile([128, 512], BF16, tag="e")
            nc.scalar.activation(e[:], psl2[:], ACTF.Exp, bias=bias_s[:],
                                 scale=1.0 / ASCALE)
            ps4 = ps_sm.tile([4, 512], F32, tag="sm")
            nc.tensor.matmul(ps4[:], gs_s[:], e[:])
            # ~18-bit reciprocal straight to bf16 (wrapper insists on fp32
            # out; the NR result casts on the write port)
            from concourse.dve_ops import (
                RECIP_APPROX_FAST_CONSTS as _RC,
                RECIPROCAL_APPROX_FAST as _RF,
            )
            r4 = sb.tile([4, 512], BF16, tag="r4")
            nc.vector._custom_dve(
                _RF, out=r4[:], in0=ps4[:],
                s0=_RC["s0"], s1=_RC["s1"], imm2=_RC["imm2"],
            )
            pR = ps_pr.tile([128, 512], F32, tag="pr")
            nc.tensor.matmul(pR[:], gb_s[:], r4[:])
            wt = sb.tile([128, 512], BF16, tag="wt")
            nc.vector.tensor_tensor(wt[:], e[:], pR[:], ALU.mult)

            if uu == 0:
                pwx[b_] = ps_wx.tile([32, XTW], F32, tag="wx", name="pwx")

            pwtT = ps_wtt.tile([128, 512], BF16, tag="wtt")
            for sl in range(4):
                nc.tensor.transpose(
                    pwtT[:, 128 * sl:128 * sl + 128],
                    wt[:, 128 * sl:128 * sl + 128],
                    idt_s[:],
                )
            wtTs = sb.tile([128, 512], BF16, tag="wtTs")
            nc.vector.tensor_copy(wtTs[:], pwtT[:])
            for j in range(NCHUNK):
                nc.tensor.matmul(
                    pwx[b_][:, 0:XTW],
                    wtTs[:, 128 * (j % 4) + 32 * (j // 4):
                         128 * (j % 4) + 32 * (j // 4) + 32],
                    xT[:, XTW * j:XTW * (j + 1)],
                    start=(uu == 0 and j == 0),
                    stop=(uu == 1 and j == NCHUNK - 1),
                    skip_group_check=True,
                )
            if uu == 1:
                outs = sb.tile([32, 256], F32, tag="outs")
                nc.vector.scalar_tensor_tensor(
                    out=outs[:], in0=cw_s[:], scalar=pwx[b_][:, 256:257],
                    in1=pwx[b_][:, 0:256], op0=ALU.mult, op1=ALU.add,
                )
                nc.gpsimd.dma_start(out_d[b_], outs[:])
                del pwx[b_]

        # unit-0/1 loads hit the DMA queue before the constants (the x
        # data is the critical path at startup); loads then run 2 units
        # ahead, and stage_b(u-1) is emitted before stage_a(u) so the
        # softmax chain outranks the next unit's work in every engine's
        # priority queue.
        load_consts()
        loads = [stage_load(0), stage_load(1), stage_load(2)]
        prev = stage_a(loads[0])
        for u in range(1, units):
            if u + 2 < units:
                loads.append(stage_load(u + 2))
            stage_b(prev)
            prev = stage_a(loads[u])
        stage_b(prev)

    nc.finalize()
    return nc


def host_constants(codewords, scale):
    cw = np.asarray(codewords, dtype=np.float32)
    sc = np.asarray(scale, dtype=np.float32)
    c_sq = (cw.astype(np.float64) ** 2).sum(-1).astype(np.float32)

    # A[g, p, h, m]: fp8 DoubleRow layout — contraction pair (p, h)
    # covers channel h*128+p; rescaled by ASCALE for e4m3 range.
    A = np.zeros((NGRP, 128, 2, 128), np.float32)
    for cc in range(2):
        blk = ASCALE * (-2.0 * sc[None, :]) * cw[:, cc * 128:(cc + 1) * 128].T
        for g in range(NGRP):
            A[g, :, cc, 32 * g:32 * g + 32] = blk

    SG = np.zeros((NGRP, 128, 128), np.float32)
    BIASB = np.zeros((128, 1), np.float32)
    GS = np.zeros((128, 4), np.float32)
    GB = np.zeros((4, 128), np.float32)
    for g in range(4):
        SG[g, :, 32 * g:32 * g + 32] = (ASCALE * sc)[None, :]
        BIASB[32 * g:32 * g + 32, 0] = sc * c_sq + 8.0
        GS[32 * g:32 * g + 32, g] = 1.0
        GB[g, 32 * g:32 * g + 32] = 1.0

    # device layouts: A (p, g, h, m), SG (p, g, m) -- contiguous DMAs
    A = np.ascontiguousarray(A.transpose(1, 0, 2, 3))
    SG = np.ascontiguousarray(SG.transpose(1, 0, 2))
    return {
        "A": A.astype(F8), "SG": SG.astype(np.float16), "BIASB": BIASB,
        "GS": GS.astype(BF), "GB": GB.astype(BF),
        "CWD": np.ascontiguousarray(-cw),
        "IDT": np.eye(128, dtype=BF),
    }


_CACHE = {}


def pack_x(x):
    """Host marshaling: bf16 natural + bf16 pre-transposed layouts."""
    xb = x.reshape(B, 2, 128, HW).astype(BF)        # (b, cc, p, t)
    xn = np.ascontiguousarray(
        x.reshape(B, 2, 128, HW).astype(F8).transpose(0, 2, 1, 3))
    # transposed: (b, chunk, i, c) with ones/pad cols, then unit-major
    xt = np.empty((B, HW // 128, 128, XTW), dtype=BF)
    xt[..., 256] = 1.0
    xt[..., 257] = 0.0
    # (b, cc, p, ch, i) -> (b, ch, i, cc*128+p)
    xt[..., 0:256] = (
        xb.reshape(B, 2, 128, HW // 128, 128)
        .transpose(0, 3, 4, 1, 2)
        .reshape(B, HW // 128, 128, 256))
    # (b, ch, i, c) -> (unit, j, i, c) -> (unit, i, j*c)
    xt = xt.reshape(B * HW // UNIT, NCHUNK, 128, XTW).transpose(0, 2, 1, 3)
    xt = np.ascontiguousarray(xt.reshape(B * HW // UNIT, 128, NCHUNK * XTW))
    return xn, xt


def make_in_maps(inputs):
    x = np.asarray(inputs["x"], dtype=np.float32)
    consts = host_constants(inputs["codewords"], inputs["scale"])
    xn, xt = pack_x(x)
    upc = BL * HW // UNIT   # units per core
    in_maps = []
    for i in range(N_CORES):
        m = dict(consts)
        m["XN"] = np.ascontiguousarray(xn[BL * i:BL * (i + 1)])
        m["XT"] = np.ascontiguousarray(xt[upc * i:upc * (i + 1)])
        in_maps.append(m)
    return in_maps


def kernel(x, codewords, scale):
    if "nc" not in _CACHE:
        _CACHE["nc"] = build_module()
    nc = _CACHE["nc"]
    in_maps = make_in_maps(dict(x=x, codewords=codewords, scale=scale))
    res = run_bass_kernel_spmd(nc, in_maps, list(range(N_CORES)))
    out = np.concatenate([r["out"] for r in res.results], axis=0)
    return out.astype(np.float32)
